# revision 48
# baseline (speedup 1.0000x reference)
"""Multi-head attention kernel for Trainium2, sharded over 8 NeuronCores.

Problem: q,k,v [2, 4096, 256], 8 heads of d=32.  b*h = 16 head-instances
are sharded 2-per-core (core c -> batch c//4, head-pair c%4); no
cross-core communication.

The softmax exp is the fundamental per-core bottleneck: 33.5M exps must
each pass exactly once through a PSUM-reading engine (ScalarE or
VectorE, both 1 elem/cycle/lane from fp32 PSUM; GPSIMD/DMA have no PSUM
port).  The kernel splits chunks across BOTH engines (kc parity):

  - ScalarE: real exp via ACTIVATE (its free affine undoes the
    matmul-side Schraudolph scaling), fp16 out.
  - VectorE: a custom 8-stage DVE op (MNT_EXP_ANT) computing a
    mantissa-corrected Schraudolph exp: the QK^T matmul pre-scales K by
    A=1024*log2(e)/sqrt(d) and adds B=15360 via a 33rd contraction row,
    so PSUM holds y = 1024*(log2 e)*s + 15360.  The op computes
    round(y) + quadratic-mantissa-correction (bit-domain AND-mask
    extracts the low 10 bits f; r=(f*cq+b1)*f fixes the log-linear
    mantissa) and writes int16; those bits reinterpreted as fp16 ARE
    exp(s) to ~0.1%.  Runs at the same 1x PSUM-read rate as a plain
    copy: exp at copy cost.  End-to-end rel err ~1.5e-3.

Main loop (per q-tile of 512, per k-chunk of 128): S matmuls for the
two heads use separate 64-row array tiles ((0,0)/(64,0), C=33 with the
bias row); PV matmuls use separate 64-col tiles ((0,0)/(0,64), M=33
with the [V|1] ones-column accumulating the softmax denominator); PV
lags the exp stage by `lag` k-chunks so its engine dependency never
head-of-line-blocks the in-order PE queue.  The epilogue only copies
the raw O^T strips + denominator rows to SBUF and DMAs them out;
transpose and normalization happen on the host in kernel().
"""

import numpy as np

import concourse.mybir as mybir
import concourse.tile as tile
from concourse import bacc, bass_utils
from concourse.masks import make_identity

B, N, C, H, D = 2, 4096, 256, 8, 32
NCORES = 8
HPC = 2                      # heads per core
COLS = HPC * D               # 64 per-core channel columns
P = 128                      # partitions / k-chunk
QTILE = 512                  # q columns per head per PSUM group
NKC = N // P                 # 32 k-chunks
NQT = N // QTILE             # 8 q-tiles per head
QTG = 2                      # q-tiles per group (paired: ScalarE/VectorE)
NQG = NQT // QTG             # 4 groups
F32 = mybir.dt.float32
BF16 = mybir.dt.float16      # fp16: same PE speed as bf16, finer mantissa
I16 = mybir.dt.int16
U32 = mybir.dt.uint32

# Schraudolph constants: PSUM holds y = AEXP*s + BIAS (s = q.k/sqrt(d));
# A (folded into kT) also absorbs the 1/sqrt(d) attention scale.
AEXP = float(1024.0 * np.log2(np.e))
A = float(AEXP / np.sqrt(D))
BIAS = 15360.0               # 15*1024: fp16 exponent bias in ticks
TWO23 = float(2.0 ** 23)
# AND-mask for t = y + 2^23 (exponent field fixed at 150): keeps sign/exp
# bits and mantissa bits 10-22, clearing the low 10.  0x4B7FFC00 is the
# FINITE float 16773120.0, so it can ride a plain fp32 immediate.
MASK = float(np.frombuffer(np.uint32(0x4B7FFC00).tobytes(), np.float32)[0])
# minimax quadratic for the mantissa: m(f) = f*(B1 + CQ*f), f in ticks
B1 = 0.665961
CQ = 0.329932 / 1024.0

_cache = {}
_mntexp = None


def _register_mntexp():
    """Register the custom DVE op (idempotent).  out_i16 bits, viewed as
    fp16, equal exp(s) given PSUM y = A*s + 15360."""
    global _mntexp
    if _mntexp is not None:
        return _mntexp
    import concourse.dve_ops as dve_ops
    from concourse.dve_spec import (
        Spec, Src0, C0, C1, C2, Bin, lower, _spill_c3_to_src1, C3,
    )
    from concourse.dve_uop import AluOp, DveOpSpec

    for op in dve_ops.OPS:
        if op.name == "MNT_EXP_ANT":
            _mntexp = op
            return op

    t = Src0 + C0                       # y + 2^23: pins binade, rounds to int
    t2 = Bin(AluOp.BITWISE_AND, t, C1)  # clear low 10 mantissa bits
    f = t - t2                          # f = round(y) mod 1024
    r = (f * C2 + C3) * f               # quadratic mantissa correction
    body = _spill_c3_to_src1((t2 + r) - C0)

    def ref(in0, in1, s0, s1, imm2):
        x = np.asarray(in0, np.float32)
        s0f = np.float32(s0 if isinstance(s0, float) else np.asarray(s0).ravel()[0])
        m = np.asarray(s1, np.float32).view(np.int32).ravel()[0]
        b1 = np.asarray(in1, np.float32).reshape(-1, 1)
        t = (x + s0f).astype(np.float32)
        t2 = (t.view(np.int32) & m).view(np.float32)
        f = (t - t2).astype(np.float32)
        r = ((f * np.float32(imm2) + b1) * f).astype(np.float32)
        return ((t2 + r) - s0f).astype(np.float32)

    spec = Spec(body=body, reference=ref)
    row = max(dve_ops._SUB_OPCODE_FOR_NAME.values()) + 1
    assert row < 0x20
    dve_ops._SUB_OPCODE_FOR_NAME["MNT_EXP_ANT"] = row
    shas = {}
    for ver in ("v3", "v4"):
        try:
            sp = DveOpSpec(name="MNT_EXP_ANT", opcode=row,
                           uops=lower(spec, ver=ver), rd1_en=True)
            shas[ver] = sp.sha(ver)
        except Exception:
            pass
    op = dve_ops.DveOp("MNT_EXP_ANT", spec, subdim=False, uops_sha=shas)
    dve_ops.OPS.append(op)
    dve_ops.CUSTOM_DVE_SPECS["MNT_EXP_ANT"] = spec
    _mntexp = op
    return op


def _emit(tc, nc, q, k, v, out, vec_mode="custom", act_mode="exp",
          split="half", do_pv=True, s_tp=True, pv_tp=True, lag=3,
          hmajor=True, pexp_bufs=8, gsz=1):
    mntexp = _register_mntexp()
    with tc.tile_pool(name="persist", bufs=1) as persist:
        ident = persist.tile([P, P], F32, name="ident")
        make_identity(nc, ident[:])
        # kT: rows 0-31 h0 k*A, row 32 bias, rows 64-95 h1 k*A, row 96 bias
        kT = persist.tile([97, N], BF16, name="kT")
        qTf = persist.tile([97, N], BF16, name="qTf")  # ones at rows 32, 96
        # V with ones column: per head, 32 chunks of [128, 33]
        vsb = persist.tile([P, HPC * NKC * (D + 1)], BF16, name="vsb")
        b1t = persist.tile([P, 1], F32, name="b1t")
        sbias = persist.tile([P, 1], F32, name="sbias")
        nc.vector.memset(b1t[:], B1)
        nc.vector.memset(sbias[:], -BIAS / AEXP)

        NST = 4                      # staging quarters (whole-tile dep unit)
        CPQ = NKC // NST             # 8 row-chunks per quarter
        with tc.tile_pool(name="stage", bufs=1) as stage_pool:
            def quarter_dma(src, name):
                # gapped staging: cols 0-31 head0, 64-95 head1 (32-63 junk)
                tiles = []
                for g in range(NST):
                    st = stage_pool.tile([P, CPQ, 96], F32, name=f"{name}{g}")
                    nc.gpsimd.memset(st[:, :, D:64], 0.0)  # junk gap
                    s3 = src.rearrange("(i p) d -> i p d", p=P)[
                        g * CPQ:(g + 1) * CPQ].rearrange("i p d -> p i d")
                    nc.sync.dma_start(st[:, :, 0:D], s3[:, :, 0:D])
                    nc.sync.dma_start(st[:, :, 64:64 + D], s3[:, :, D:2 * D])
                    tiles.append(st)
                return tiles

            kst = quarter_dma(k, "kst")
            qst = quarter_dma(q, "qst")
            vstage = stage_pool.tile([P, NKC * COLS], F32, name="vstage")
            nc.sync.dma_start(
                vstage[:].rearrange("p (i d) -> p i d", d=COLS),
                v.rearrange("(i p) d -> p i d", p=P),
            )

            with tc.tile_pool(name="tp", bufs=4, space="PSUM") as tp:
                # kT first (main loop needs all of kT for the first group),
                # 4 transposed chunks batched per PSUM->SBUF copy.
                for i4 in range(NKC // 4):
                    pt = tp.tile([96, 4 * P], F32, tag="pt")
                    for j in range(4):
                        i = i4 * 4 + j
                        g, ii = divmod(i, CPQ)
                        nc.tensor.transpose(
                            pt[:, j * P:(j + 1) * P], kst[g][:, ii, :],
                            ident[:],
                        )
                    nc.scalar.mul(
                        kT[0:96, i4 * 4 * P:(i4 + 1) * 4 * P], pt[:], A
                    )
                for i4 in range(NKC // 4):
                    pt = tp.tile([96, 4 * P], F32, tag="pt")
                    for j in range(4):
                        i = i4 * 4 + j
                        g, ii = divmod(i, CPQ)
                        nc.tensor.transpose(
                            pt[:, j * P:(j + 1) * P], qst[g][:, ii, :],
                            ident[:],
                        )
                    nc.vector.tensor_copy(
                        qTf[0:96, i4 * 4 * P:(i4 + 1) * 4 * P], pt[:]
                    )
                # bias / ones rows (GPSIMD: keeps ScalarE/VectorE free)
                nc.gpsimd.memset(kT[32:33, :], BIAS)
                nc.gpsimd.memset(kT[96:97, :], BIAS)
                nc.gpsimd.memset(qTf[32:33, :], 1.0)
                nc.gpsimd.memset(qTf[96:97, :], 1.0)
                # V reformat with ones column
                vv = vsb[:].rearrange("p (hh i e) -> p hh i e",
                                      hh=HPC, e=D + 1)
                vst = vstage[:].rearrange("p (i d) -> p i d", d=COLS)
                for hh in range(HPC):
                    nc.vector.tensor_copy(
                        vv[:, hh, :, 0:D], vst[:, :, hh * D:(hh + 1) * D]
                    )
                onescol = persist.tile([P, HPC * NKC], F32, name="onescol")
                nc.vector.memset(onescol[:], 1.0)
                nc.vector.tensor_copy(
                    vv[:, :, :, D],
                    onescol[:].rearrange("p (hh i) -> p hh i", hh=HPC),
                )

        # ---- main loop: groups of 2 q-tiles; per kc the qt0 chunk goes to
        # ScalarE (real exp) and the qt1 chunk to VectorE (custom op). ----
        with (
            tc.tile_pool(name="ps", bufs=3, space="PSUM") as ps_pool,
            tc.tile_pool(name="po", bufs=2, space="PSUM") as po_pool,
            tc.tile_pool(name="pexp", bufs=pexp_bufs) as pexp_pool,
            tc.tile_pool(name="osb", bufs=4) as osb_pool,
        ):
            def emit_S1(ps, qt, kc):
                # 64-row 2-tile mode: h0 rows 0-32 (T0), h1 rows 64-96 (T8)
                q0 = qt * QTILE
                for hh in range(HPC):
                    lo = 64 * hh
                    nc.tensor.matmul(
                        ps[:, hh * QTILE:(hh + 1) * QTILE],
                        lhsT=kT[lo:lo + 33, kc * P:(kc + 1) * P],
                        rhs=qTf[lo:lo + 33, q0:q0 + QTILE],
                        start=True, stop=True,
                        tile_position=(lo, 0) if s_tp else None,
                    )

            def emit_PV1(po, pexp, pkc):
                # 64-col 2-tile mode: h0 -> PSUM partitions 0-32 (T0),
                # h1 -> 64-96 (T1 at col 64) -> concurrent col-group pair.
                for hh in range(HPC):
                    vbase = hh * NKC * (D + 1)
                    vch = vsb[:, vbase + pkc * (D + 1):
                              vbase + (pkc + 1) * (D + 1)]
                    nc.tensor.matmul(
                        po[64 * hh:64 * hh + D + 1, :],
                        lhsT=vch,
                        rhs=pexp[:, hh * QTILE:(hh + 1) * QTILE],
                        start=(pkc == 0),
                        stop=(pkc == NKC - 1),
                        skip_group_check=True,
                        tile_position=(0, 64 * hh) if pv_tp else None,
                    )

            def make_epilogue(po, qt):
                # Minimal epilogue: copy the two written po strips to SBUF
                # (split across engines) and DMA the raw O^T + denominator
                # rows out; transpose + normalization happen on the host.
                osbs = {}

                def copy_osb(hh):
                    if not osbs:
                        osbs[0] = osb_pool.tile(
                            [97, QTILE], F32, tag="osb",
                            name=f"osb{qt}", uniquify=True)
                    osb = osbs[0]
                    ib = 64 * hh
                    sl = slice(ib, ib + D + 1)
                    if hh == 0:
                        nc.scalar.copy(osb[sl, :], po[sl, :])
                    else:
                        nc.vector.tensor_copy(osb[sl, :], po[sl, :])

                def dmaout(hh):
                    nc.sync.dma_start(
                        out[qt, hh * (D + 1):(hh + 1) * (D + 1), :],
                        osbs[0][64 * hh:64 * hh + D + 1, :],
                    )

                hs = list(range(HPC))
                steps = [lambda hh=hh: copy_osb(hh) for hh in hs]
                steps += [lambda hh=hh: dmaout(hh) for hh in hs]
                return steps

            if vec_mode == "none" or act_mode == "none":
                pexp_fix = pexp_pool.tile([P, HPC * QTILE], BF16,
                                          name="pexp_fix")
                nc.vector.memset(pexp_fix[:], 0.5)
            if act_mode == "sink":
                dum = pexp_pool.tile([P, HPC * QTILE], F32, name="dum")
                nc.vector.memset(dum[:], 0.5)

            def engine_op(which, ps, pexp):
                if which == "scalar":
                    if act_mode == "none":
                        return pexp_fix
                    if act_mode == "sink":
                        nc.scalar.copy(pexp[:], dum[:])
                        return pexp
                    nc.scalar.activation(
                        pexp[:], ps[:], mybir.ActivationFunctionType.Exp,
                        scale=1.0 / AEXP, bias=sbias[:],
                    )
                else:
                    if vec_mode == "none":
                        return pexp_fix
                    if vec_mode == "sink":
                        nc.vector.memset(pexp[:], 0.5)
                        return pexp
                    if vec_mode == "copy":
                        nc.vector.tensor_copy(pexp[:], ps[:])
                    else:
                        nc.vector._custom_dve(
                            mntexp, out=pexp[:].bitcast(I16), in0=ps[:],
                            in1=b1t[:], s0=TWO23, s1=MASK, imm2=CQ,
                        )
                return pexp

            pending = []
            for qt in range(NQT):
                po = po_pool.tile([97, QTILE], F32, tag="po",
                                  name=f"po_{qt}", uniquify=True)
                pvq = []          # PV lagged `lag` kc behind S/exp
                for kc in range(NKC):
                    ps = ps_pool.tile([P, HPC * QTILE], F32, tag="ps")
                    emit_S1(ps, qt, kc)
                    if pending and kc >= 1:
                        pending.pop(0)()
                    if split == "half":
                        eng = ("scalar" if ((kc // gsz) + qt) % 2 == 0
                               else "vector")
                    elif split == "scalar_all":
                        eng = "scalar"
                    else:
                        eng = "vector"
                    pexp = pexp_pool.tile([P, HPC * QTILE], BF16,
                                          tag="pexp")
                    pexp = engine_op(eng, ps, pexp)
                    pvq.append((pexp, kc))
                    if do_pv and len(pvq) > lag:
                        ppexp, pkc = pvq.pop(0)
                        emit_PV1(po, ppexp, pkc)
                if do_pv:
                    for ppexp, pkc in pvq:
                        emit_PV1(po, ppexp, pkc)
                else:
                    nc.vector.memset(po[0:97, :], 1.0)
                pvq = []
                for step in pending:   # leftovers (shouldn't happen)
                    step()
                pending = make_epilogue(po, qt)
            for step in pending:
                step()


def _build(loop=0, **emit_kw):
    """loop=0: production build.  loop>=1: body wrapped in an on-device
    For_i repeat loop (timing-only builds).  emit_kw: ablation knobs."""
    key = ("nc", loop, tuple(sorted(emit_kw.items())))
    if key in _cache:
        return _cache[key]
    nc = bacc.Bacc(
        "TRN2",
        target_bir_lowering=False,
        debug=False,
        enable_asserts=False,
        num_devices=NCORES,
    )
    q = nc.dram_tensor("q", [N, COLS], F32, kind="ExternalInput").ap()
    k = nc.dram_tensor("k", [N, COLS], F32, kind="ExternalInput").ap()
    v = nc.dram_tensor("v", [N, COLS], F32, kind="ExternalInput").ap()
    # raw per-q-tile O^T strips + denominator rows; host normalizes
    out = nc.dram_tensor("out", [NQT, HPC * (D + 1), QTILE], F32,
                         kind="ExternalOutput").ap()
    with tile.TileContext(nc) as tc:
        if loop:
            with tc.For_i(0, loop, 1):
                _emit(tc, nc, q, k, v, out, **emit_kw)
        else:
            _emit(tc, nc, q, k, v, out, **emit_kw)
    nc.compile()
    _cache[key] = nc
    return nc


def _in_maps(q, k, v):
    maps = []
    for c in range(NCORES):
        b, hp = divmod(c, 4)
        cs = slice(hp * COLS, (hp + 1) * COLS)
        maps.append({
            "q": np.ascontiguousarray(q[b, :, cs], dtype=np.float32),
            "k": np.ascontiguousarray(k[b, :, cs], dtype=np.float32),
            "v": np.ascontiguousarray(v[b, :, cs], dtype=np.float32),
        })
    return maps


def _postprocess_core(arr):
    """[NQT, 2*(D+1), QTILE] raw O^T strips -> [N, COLS] normalized."""
    a = np.asarray(arr, np.float32).reshape(NQT, HPC, D + 1, QTILE)
    o = a[:, :, :D, :]                      # [qt, hh, d, q]
    den = a[:, :, D:D + 1, :]               # [qt, hh, 1, q]
    r = (o / den).transpose(0, 3, 1, 2)     # [qt, q, hh, d]
    return np.ascontiguousarray(r.reshape(N, COLS))


def _assemble(results):
    out = np.empty((B, N, C), np.float32)
    for c in range(NCORES):
        b, hp = divmod(c, 4)
        out[b, :, hp * COLS:(hp + 1) * COLS] = _postprocess_core(
            results[c]["out"])
    return out


def kernel(q, k, v):
    nc = _build()
    res = bass_utils.run_bass_kernel_spmd(
        nc, _in_maps(q, k, v), core_ids=list(range(NCORES))
    )
    return _assemble(res.results)
